# revision 10
# baseline (speedup 1.0000x reference)
"""Trainium2 Bass kernel for the GatedBlock problem.

Computation (per row of features [N=65536, 2560] f32):
  out[0:256]      = silu(x[0:256])                       (scalar block, l=0)
  out[256:1024]   = x[256:1024]  * rep3(sigmoid(g[0:256]))    (l=1, mul=256)
  out[1024:1664]  = x[1024:1664] * rep5(sigmoid(g[256:384]))  (l=2, mul=128)
  out[1664:2112]  = x[1664:2112] * rep7(sigmoid(g[384:448]))  (l=3, mul=64)
where g = x[2112:2560]; output shape [N, 2112] f32.

Strategy: pure data parallel over 8 NeuronCores (8192 rows each). The
problem is HBM-bound (~358 GB/s per core), so I/O is done in fp16: the
host casts f32->f16 before upload and upcasts the f16 result, halving
HBM traffic (76.6 MB/core vs 153 MB/core in f32). fp16 keeps relative
error ~1e-3, far inside the 2e-2 gate. Row-tiles of 128 partitions x R
rows/partition; sigmoid of the gates and the l=0 silu both run on the
scalar (ACT) engine, gating multiplies on the vector engine with the
gate broadcast along the (2l+1) fastest dim. Loads on the sync HWDGE
ring, stores on the scalar HWDGE ring so the two streams interleave.
"""

from contextlib import ExitStack

import numpy as np

import concourse.bacc as bacc
import concourse.bass as bass
import concourse.tile as tile
from concourse import mybir
from concourse.bass_utils import run_bass_kernel_spmd

P = 128
FEAT = 2560
SIZE_OUT = 2112
N_GATES = 448
SCALAR_D = 256  # l=0 block width (silu)
GATED_BLOCKS = [(256, 1), (128, 2), (64, 3)]  # (mul, l) for l>0 blocks

N_CORES = 8
N_ROWS = 65536
ROWS_PER_CORE = N_ROWS // N_CORES

F16 = mybir.dt.float16
F32 = mybir.dt.float32
SIGMOID = mybir.ActivationFunctionType.Sigmoid
SILU = mybir.ActivationFunctionType.Silu

# Production configuration (used by kernel() and test.py's timing path).
NP_DTYPE = np.float16
BUILD_KWARGS = dict(
    R=4,
    dtype=F16,
    pool_bufs=(4, 4, 4),
    inplace=False,
    load_eng="sync",
    store_eng="scalar",
)


def build_program(
    rows: int,
    R: int = 4,  # rows per partition per tile
    reps: int = 1,  # timing: repeat whole body in a HW loop
    dtype=F16,
    pool_bufs: tuple = (4, 4, 4),  # (xin, yout, sig) buffer counts
    inplace: bool = False,  # multiply into xt and store from it (no yout pool)
    load_eng: str = "sync",
    store_eng: str = "scalar",
    body_reps: int = 1,  # timing: unrolled copies of the body per For_i iter
    dma_group: int = 1,  # tiles moved per DMA instruction (bigger transfers)
    sg_inplace: bool = False,  # sigmoid written back into xt's gate region
) -> bass.Bass:
    g = dma_group
    rows_per_tile = P * R * g
    assert rows % rows_per_tile == 0
    n_tiles = rows // rows_per_tile

    nc = bacc.Bacc("TRN2", target_bir_lowering=False, debug=False)
    x = nc.dram_tensor("x", [rows, FEAT], dtype, kind="ExternalInput")
    y = nc.dram_tensor("y", [rows, SIZE_OUT], dtype, kind="ExternalOutput")
    xv = x.ap().rearrange("(t s p r) c -> t p s r c", s=g, p=P, r=R)
    yv = y.ap().rearrange("(t s p r) c -> t p s r c", s=g, p=P, r=R)

    def eng(spec: str, t: int):
        if spec == "alt":
            spec = "sync" if t % 2 == 0 else "scalar"
        elif spec == "alt2":
            spec = "scalar" if t % 2 == 0 else "sync"
        return getattr(nc, spec)

    def body(tc):
        for t in range(n_tiles):
            xt = xpool.tile([P, g, R, FEAT], dtype)
            if load_eng == "split":
                h = FEAT // 2
                nc.sync.dma_start(out=xt[:, :, :, 0:h], in_=xv[t][:, :, :, 0:h])
                nc.scalar.dma_start(
                    out=xt[:, :, :, h:FEAT], in_=xv[t][:, :, :, h:FEAT]
                )
            else:
                eng(load_eng, t).dma_start(out=xt, in_=xv[t])

            yt = xt if inplace else ypool.tile([P, g, R, SIZE_OUT], dtype)
            sg = (
                xt[:, :, :, SIZE_OUT:FEAT]
                if sg_inplace
                else spool.tile([P, g, R, N_GATES], dtype)
            )
            for s in range(g):
                xs, ys, sgs = xt[:, s], yt[:, s], sg[:, s]
                # sigmoid of the gates + l=0 silu on the scalar engine
                nc.scalar.activation(
                    out=sgs, in_=xs[:, :, SIZE_OUT:FEAT], func=SIGMOID
                )
                nc.scalar.activation(
                    out=ys[:, :, 0:SCALAR_D], in_=xs[:, :, 0:SCALAR_D], func=SILU
                )
                off, goff = SCALAR_D, 0
                for mul, l in GATED_BLOCKS:
                    d = 2 * l + 1
                    # [P, R, mul, d] view; gate broadcast over fastest dim d
                    yb = ys[:, :, off : off + mul * d].rearrange(
                        "p r (m d) -> p r m d", d=d
                    )
                    xb = xs[:, :, off : off + mul * d].rearrange(
                        "p r (m d) -> p r m d", d=d
                    )
                    gb = (
                        sgs[:, :, goff : goff + mul]
                        .unsqueeze(3)
                        .broadcast_to([P, R, mul, d])
                    )
                    nc.vector.tensor_mul(yb, xb, gb)
                    off += mul * d
                    goff += mul

            st = yt[:, :, :, 0:SIZE_OUT] if inplace else yt
            eng(store_eng, t).dma_start(out=yv[t], in_=st)

    xb, yb, sb = pool_bufs
    with tile.TileContext(nc) as tc, ExitStack() as ctx:
        xpool = ctx.enter_context(tc.tile_pool(name="xin", bufs=xb))
        ypool = None if inplace else ctx.enter_context(
            tc.tile_pool(name="yout", bufs=yb))
        spool = None if sg_inplace else ctx.enter_context(
            tc.tile_pool(name="sig", bufs=sb))
        if reps == 1:
            for _ in range(body_reps):
                body(tc)
        else:
            with tc.For_i(0, reps, 1):
                for _ in range(body_reps):
                    body(tc)
    nc.finalize()
    return nc


_PROGRAM_CACHE: dict = {}


def _get_program() -> bass.Bass:
    key = "production"
    if key not in _PROGRAM_CACHE:
        _PROGRAM_CACHE[key] = build_program(ROWS_PER_CORE, **BUILD_KWARGS)
    return _PROGRAM_CACHE[key]


def kernel(features: np.ndarray) -> np.ndarray:
    assert features.shape == (N_ROWS, FEAT), features.shape
    features = np.asarray(features, dtype=NP_DTYPE)
    nc = _get_program()
    shards = np.split(features, N_CORES, axis=0)
    in_maps = [{"x": np.ascontiguousarray(s)} for s in shards]
    res = run_bass_kernel_spmd(nc, in_maps, list(range(N_CORES)))
    out = np.concatenate([res.results[i]["y"] for i in range(N_CORES)], axis=0)
    return out.astype(np.float32)
